# revision 1
# baseline (speedup 1.0000x reference)
"""Trainium2 Bass kernel for nn_AttnCoef (sparse attention coefficients).

Problem: alpha = softmax_masked(q @ k^T / sqrt(DH)) over Lk = n^2, with an
all-distinct index mask M(i,(j,k)) = [i!=j][i!=k][j!=k] and node-validity
masks. Output [H=4, B=4, Lq=128, Lk=16384] f32 (128 MiB).

Strategy (8 NeuronCores, data parallel over the 16 (h,b) pairs, 2 per core):
- All masking is folded into the matmul as additive -C biases so that
  exp() underflows masked entries to exactly 0:
    * lk-only mask ([j!=k] & node masks) rides an extra contraction row
      (weight 1.0, bias row -C*(1-kvalid)).
    * [i=j] block mask: 4 extra contraction rows with per-chunk one-hot
      weights (chunk = 512 lk = 4 j-blocks).
    * [i=k] strided diagonal mask: second accumulating matmul
      (-C*I as stationary, periodic identity as moving operand).
- One ScalarE pass: p = exp(0.25*psum) PSUM->SBUF (bf16) with fused
  per-row accumulation (denominators).
- recip = q_mask / (denom + eps); one VectorE tensor_scalar pass scales
  p by recip into f32; DMA out.
"""

import sys

sys.path.insert(0, "/opt/trn_rl_repo")

import numpy as np
import ml_dtypes

H, B, N, DQK, DH = 4, 4, 128, 64, 16
LK = N * N  # 16384
NCORES = 8
PAIRS_PER_CORE = 2
NCHUNK, CW = 32, 512  # matmul chunks per pair
NGRP, GW = 8, 2048  # psum groups per pair (4 chunks each)
KDIM = DH + 1 + 4  # 21 contraction rows
BIGC = 98304.0  # additive mask constant (exact in bf16/f32)

TRACE = False
_LAST = None
_NC_CACHE = None


def _build_nc():
    import concourse.tile as tile
    from concourse import bacc, mybir

    nc = bacc.Bacc(None, target_bir_lowering=False)
    f32, bf16 = mybir.dt.float32, mybir.dt.bfloat16

    lhs_e = nc.declare_dram_parameter(
        "lhs", [KDIM, PAIRS_PER_CORE, NCHUNK, N], f32, isOutput=False
    )
    rhs_e = nc.declare_dram_parameter(
        "rhs", [PAIRS_PER_CORE, KDIM, LK], f32, isOutput=False
    )
    wid_e = nc.declare_dram_parameter("wid", [N, N], bf16, isOutput=False)
    irep_e = nc.declare_dram_parameter("irep", [N, CW], bf16, isOutput=False)
    qm_e = nc.declare_dram_parameter("qmask", [N, PAIRS_PER_CORE], f32, isOutput=False)
    out_e = nc.declare_dram_parameter(
        "out", [PAIRS_PER_CORE * N, LK], f32, isOutput=True
    )

    EXP = mybir.ActivationFunctionType.Exp
    ADD = mybir.AluOpType.add
    AXX = mybir.AxisListType.X

    with tile.TileContext(nc) as tc:
        with (
            tc.tile_pool(name="consts", bufs=1) as consts,
            tc.tile_pool(name="rhsp", bufs=1) as rhsp,
            tc.tile_pool(name="pp", bufs=2) as pp,
            tc.tile_pool(name="psum", bufs=2, space="PSUM") as psum,
            tc.tile_pool(name="small", bufs=2) as small,
            tc.tile_pool(name="op", bufs=2) as op,
        ):
            lhs_t = consts.tile([KDIM, PAIRS_PER_CORE, NCHUNK, N], f32)
            nc.sync.dma_start(out=lhs_t[:], in_=lhs_e[:])
            wid_t = consts.tile([N, N], bf16)
            nc.sync.dma_start(out=wid_t[:], in_=wid_e[:])
            irep_t = consts.tile([N, CW], bf16)
            nc.sync.dma_start(out=irep_t[:], in_=irep_e[:])
            qm_t = consts.tile([N, PAIRS_PER_CORE], f32)
            nc.sync.dma_start(out=qm_t[:], in_=qm_e[:])

            out_ap = out_e[:]

            for u in range(PAIRS_PER_CORE):
                rhs_t = rhsp.tile([KDIM, LK], f32, tag="rhs")
                nc.sync.dma_start(out=rhs_t[:], in_=rhs_e[:][u])

                p_t = pp.tile([N, LK], bf16, tag="p")
                dsum = small.tile([N, NGRP], f32, tag="dsum")

                for g in range(NGRP):
                    ps = psum.tile([N, GW], f32, tag="ps")
                    for cc in range(4):
                        c = 4 * g + cc
                        nc.tensor.matmul(
                            ps[:, cc * CW : (cc + 1) * CW],
                            lhs_t[:, u, c, :],
                            rhs_t[:, c * CW : (c + 1) * CW],
                            start=True,
                            stop=False,
                        )
                    for cc in range(4):
                        nc.tensor.matmul(
                            ps[:, cc * CW : (cc + 1) * CW],
                            wid_t[:],
                            irep_t[:],
                            start=False,
                            stop=True,
                        )
                    nc.scalar.activation(
                        out=p_t[:, g * GW : (g + 1) * GW],
                        in_=ps[:],
                        func=EXP,
                        scale=0.25,
                        accum_out=dsum[:, g : g + 1],
                    )

                den = small.tile([N, 1], f32, tag="den")
                nc.vector.tensor_reduce(out=den, in_=dsum[:], axis=AXX, op=ADD)
                den2 = small.tile([N, 1], f32, tag="den2")
                nc.vector.tensor_scalar_add(out=den2, in0=den, scalar1=1e-30)
                recip = small.tile([N, 1], f32, tag="recip")
                nc.vector.reciprocal(out=recip, in_=den2)
                recipf = small.tile([N, 1], f32, tag="recipf")
                nc.vector.tensor_mul(out=recipf, in0=recip, in1=qm_t[:, u : u + 1])

                for g in range(NGRP):
                    ob = op.tile([N, GW], f32, tag="ob")
                    nc.vector.tensor_scalar_mul(
                        out=ob, in0=p_t[:, g * GW : (g + 1) * GW], scalar1=recipf
                    )
                    nc.sync.dma_start(
                        out=out_ap[u * N : (u + 1) * N, g * GW : (g + 1) * GW],
                        in_=ob[:],
                    )

    nc.compile()
    return nc


def _host_inputs(q_A, k_A, q_mask, k_mask):
    q_A = np.ascontiguousarray(np.asarray(q_A, dtype=np.float32))
    k_A = np.ascontiguousarray(np.asarray(k_A, dtype=np.float32))
    q_mask = np.asarray(q_mask).astype(bool)
    k_mask = np.asarray(k_mask).astype(bool)

    # [h, b, d, i] and [h, b, d, lk]
    qt = q_A.reshape(B, N, H, DH).transpose(2, 0, 3, 1)
    kt = k_A.reshape(B, LK, H, DH).transpose(2, 0, 3, 1)

    jne = ~np.eye(N, dtype=bool)
    kvalid = (k_mask & jne[None]).reshape(B, LK)  # [b, lk]
    row16 = (-BIGC) * (~kvalid).astype(np.float32)  # [b, lk]

    lk = np.arange(LK)
    # j-block bias rows (periodic in chunks of 512): -C where (lk//128)%4 == t
    jpat = np.where(
        ((lk // N) % 4)[None, :] == np.arange(4)[:, None], -BIGC, 0.0
    ).astype(np.float32)  # [4, LK]
    # per-chunk one-hot weights for the j rows: 1.0 where i == 4c+t
    ii = np.arange(N)
    cidx = np.arange(NCHUNK)
    lhsaug = (
        (ii[None, None, :] == (4 * cidx[None, :, None] + np.arange(4)[:, None, None]))
    ).astype(np.float32)  # [4, 32, 128]

    wid = (-BIGC * np.eye(N, dtype=np.float32)).astype(ml_dtypes.bfloat16)
    irep = (np.arange(CW) % N == np.arange(N)[:, None]).astype(ml_dtypes.bfloat16)

    in_maps = []
    for core in range(NCORES):
        lhs_arr = np.zeros((KDIM, PAIRS_PER_CORE, NCHUNK, N), np.float32)
        rhs_arr = np.empty((PAIRS_PER_CORE, KDIM, LK), np.float32)
        qm_arr = np.zeros((N, PAIRS_PER_CORE), np.float32)
        for u in range(PAIRS_PER_CORE):
            P = PAIRS_PER_CORE * core + u
            h, b = P // B, P % B
            lhs_arr[0:DH, u, :, :] = qt[h, b][:, None, :]
            lhs_arr[DH, u, :, :] = 1.0
            lhs_arr[DH + 1 :, u, :, :] = lhsaug
            rhs_arr[u, 0:DH] = kt[h, b]
            rhs_arr[u, DH] = row16[b]
            rhs_arr[u, DH + 1 :] = jpat
            qm_arr[:, u] = q_mask[b].astype(np.float32)
        in_maps.append(
            {
                "lhs": lhs_arr,
                "rhs": rhs_arr,
                "wid": wid,
                "irep": irep,
                "qmask": qm_arr,
            }
        )
    return in_maps


def kernel(q_A, k_A, q_mask, k_mask):
    global _NC_CACHE, _LAST
    from concourse.bass_utils import run_bass_kernel_spmd

    if _NC_CACHE is None:
        _NC_CACHE = _build_nc()
    nc = _NC_CACHE

    in_maps = _host_inputs(q_A, k_A, q_mask, k_mask)
    res = run_bass_kernel_spmd(
        nc, in_maps, core_ids=list(range(NCORES)), trace=TRACE
    )
    _LAST = res

    alpha = np.empty((H, B, N, LK), np.float32)
    for core in range(NCORES):
        o = res.results[core]["out"]
        for u in range(PAIRS_PER_CORE):
            P = PAIRS_PER_CORE * core + u
            alpha[P // B, P % B] = o[u * N : (u + 1) * N]
    return alpha


# revision 2
# speedup vs baseline: 1.6772x; 1.6772x over previous
"""Trainium2 Bass kernel for nn_AttnCoef (sparse attention coefficients).

Problem: alpha = softmax_masked(q @ k^T / sqrt(DH)) over Lk = n^2, with an
all-distinct index mask M(i,(j,k)) = [i!=j][i!=k][j!=k] and node-validity
masks. Output [H=4, B=4, Lq=128, Lk=16384] f32 (128 MiB).

Strategy (8 NeuronCores, data parallel over the 16 (h,b) pairs, 2 per core):
- All masking is folded into the matmul as additive -C biases so that
  exp() underflows masked entries to exactly 0:
    * lk-only mask ([j!=k] & node masks) rides an extra contraction row
      (weight 1.0, bias row -C*(1-kvalid)).
    * [i=j] block mask: 4 extra contraction rows with per-chunk one-hot
      weights (chunk = 512 lk = 4 j-blocks).
    * [i=k] strided diagonal mask: second accumulating matmul
      (-C*I as stationary, periodic identity as moving operand).
- One ScalarE pass: p = exp(0.25*psum) PSUM->SBUF (bf16) with fused
  per-row accumulation (denominators).
- recip = q_mask / (denom + eps); one VectorE tensor_scalar pass scales
  p by recip into f32; DMA out.
"""

import sys

sys.path.insert(0, "/opt/trn_rl_repo")

import numpy as np
import ml_dtypes

H, B, N, DQK, DH = 4, 4, 128, 64, 16
LK = N * N  # 16384
NCORES = 8
PAIRS_PER_CORE = 2
NCHUNK, CW = 32, 512  # matmul chunks per pair
NGRP, GW = 8, 2048  # psum groups per pair (4 chunks each)
NOUT, OW = 4, 4096  # output store chunks per pair
KDIM = DH + 1 + 4  # 21 contraction rows
BIGC = 98304.0  # additive mask constant (exact in bf16/f32)

TRACE = False
_LAST = None
_NC_CACHE = None


def _build_nc():
    import concourse.tile as tile
    from concourse import bacc, mybir

    nc = bacc.Bacc(None, target_bir_lowering=False)
    f32, bf16 = mybir.dt.float32, mybir.dt.bfloat16

    lhs_e = nc.declare_dram_parameter(
        "lhs", [KDIM, PAIRS_PER_CORE, NCHUNK, N], bf16, isOutput=False
    )
    rhs_e = nc.declare_dram_parameter(
        "rhs", [PAIRS_PER_CORE, KDIM, LK], bf16, isOutput=False
    )
    wid_e = nc.declare_dram_parameter("wid", [N, N], bf16, isOutput=False)
    irep_e = nc.declare_dram_parameter("irep", [N, CW], bf16, isOutput=False)
    qm_e = nc.declare_dram_parameter("qmask", [N, PAIRS_PER_CORE], f32, isOutput=False)
    out_e = nc.declare_dram_parameter(
        "out", [PAIRS_PER_CORE * N, LK], f32, isOutput=True
    )

    EXP = mybir.ActivationFunctionType.Exp
    ADD = mybir.AluOpType.add
    AXX = mybir.AxisListType.X

    with tile.TileContext(nc) as tc:
        with (
            tc.tile_pool(name="consts", bufs=1) as consts,
            tc.tile_pool(name="rhsp", bufs=1) as rhsp,
            tc.tile_pool(name="pp", bufs=2) as pp,
            tc.tile_pool(name="psum", bufs=2, space="PSUM") as psum,
            tc.tile_pool(name="small", bufs=2) as small,
            tc.tile_pool(name="op", bufs=2) as op,
        ):
            lhs_t = consts.tile([KDIM, PAIRS_PER_CORE, NCHUNK, N], bf16)
            nc.sync.dma_start(out=lhs_t[:], in_=lhs_e[:])
            wid_t = consts.tile([N, N], bf16)
            nc.sync.dma_start(out=wid_t[:], in_=wid_e[:])
            irep_t = consts.tile([N, CW], bf16)
            nc.sync.dma_start(out=irep_t[:], in_=irep_e[:])
            qm_t = consts.tile([N, PAIRS_PER_CORE], f32)
            nc.sync.dma_start(out=qm_t[:], in_=qm_e[:])

            out_ap = out_e[:]

            for u in range(PAIRS_PER_CORE):
                rhs_t = rhsp.tile([KDIM, LK], bf16, tag="rhs")
                nc.sync.dma_start(out=rhs_t[:], in_=rhs_e[:][u])

                p_t = pp.tile([N, LK], bf16, tag="p")
                dsum = small.tile([N, NGRP], f32, tag="dsum")

                for g in range(NGRP):
                    ps = psum.tile([N, GW], f32, tag="ps")
                    for cc in range(4):
                        c = 4 * g + cc
                        nc.tensor.matmul(
                            ps[:, cc * CW : (cc + 1) * CW],
                            lhs_t[:, u, c, :],
                            rhs_t[:, c * CW : (c + 1) * CW],
                            start=True,
                            stop=False,
                        )
                    for cc in range(4):
                        nc.tensor.matmul(
                            ps[:, cc * CW : (cc + 1) * CW],
                            wid_t[:],
                            irep_t[:],
                            start=False,
                            stop=True,
                        )
                    nc.scalar.activation(
                        out=p_t[:, g * GW : (g + 1) * GW],
                        in_=ps[:],
                        func=EXP,
                        scale=0.25,
                        accum_out=dsum[:, g : g + 1],
                    )

                den = small.tile([N, 1], f32, tag="den")
                nc.vector.tensor_reduce(out=den, in_=dsum[:], axis=AXX, op=ADD)
                den2 = small.tile([N, 1], f32, tag="den2")
                nc.vector.tensor_scalar_add(out=den2, in0=den, scalar1=1e-30)
                recip = small.tile([N, 1], f32, tag="recip")
                nc.vector.reciprocal(out=recip, in_=den2)
                recipf = small.tile([N, 1], f32, tag="recipf")
                nc.vector.tensor_mul(out=recipf, in0=recip, in1=qm_t[:, u : u + 1])

                for g in range(NOUT):
                    ob = op.tile([N, OW], f32, tag="ob")
                    nc.vector.tensor_scalar_mul(
                        out=ob, in0=p_t[:, g * OW : (g + 1) * OW], scalar1=recipf
                    )
                    nc.sync.dma_start(
                        out=out_ap[u * N : (u + 1) * N, g * OW : (g + 1) * OW],
                        in_=ob[:],
                    )

    nc.compile()
    return nc


def _host_inputs(q_A, k_A, q_mask, k_mask):
    q_A = np.ascontiguousarray(np.asarray(q_A, dtype=np.float32))
    k_A = np.ascontiguousarray(np.asarray(k_A, dtype=np.float32))
    q_mask = np.asarray(q_mask).astype(bool)
    k_mask = np.asarray(k_mask).astype(bool)

    # [h, b, d, i] and [h, b, d, lk]
    qt = q_A.reshape(B, N, H, DH).transpose(2, 0, 3, 1)
    kt = k_A.reshape(B, LK, H, DH).transpose(2, 0, 3, 1)

    jne = ~np.eye(N, dtype=bool)
    kvalid = (k_mask & jne[None]).reshape(B, LK)  # [b, lk]
    row16 = (-BIGC) * (~kvalid).astype(np.float32)  # [b, lk]

    lk = np.arange(LK)
    # j-block bias rows (periodic in chunks of 512): -C where (lk//128)%4 == t
    jpat = np.where(
        ((lk // N) % 4)[None, :] == np.arange(4)[:, None], -BIGC, 0.0
    ).astype(np.float32)  # [4, LK]
    # per-chunk one-hot weights for the j rows: 1.0 where i == 4c+t
    ii = np.arange(N)
    cidx = np.arange(NCHUNK)
    lhsaug = (
        (ii[None, None, :] == (4 * cidx[None, :, None] + np.arange(4)[:, None, None]))
    ).astype(np.float32)  # [4, 32, 128]

    wid = (-BIGC * np.eye(N, dtype=np.float32)).astype(ml_dtypes.bfloat16)
    irep = (np.arange(CW) % N == np.arange(N)[:, None]).astype(ml_dtypes.bfloat16)

    in_maps = []
    for core in range(NCORES):
        lhs_arr = np.zeros((KDIM, PAIRS_PER_CORE, NCHUNK, N), ml_dtypes.bfloat16)
        rhs_arr = np.empty((PAIRS_PER_CORE, KDIM, LK), ml_dtypes.bfloat16)
        qm_arr = np.zeros((N, PAIRS_PER_CORE), np.float32)
        for u in range(PAIRS_PER_CORE):
            P = PAIRS_PER_CORE * core + u
            h, b = P // B, P % B
            lhs_arr[0:DH, u, :, :] = qt[h, b][:, None, :]
            lhs_arr[DH, u, :, :] = 1.0
            lhs_arr[DH + 1 :, u, :, :] = lhsaug
            rhs_arr[u, 0:DH] = kt[h, b]
            rhs_arr[u, DH] = row16[b]
            rhs_arr[u, DH + 1 :] = jpat
            qm_arr[:, u] = q_mask[b].astype(np.float32)
        in_maps.append(
            {
                "lhs": lhs_arr,
                "rhs": rhs_arr,
                "wid": wid,
                "irep": irep,
                "qmask": qm_arr,
            }
        )
    return in_maps


def kernel(q_A, k_A, q_mask, k_mask):
    global _NC_CACHE, _LAST
    from concourse.bass_utils import run_bass_kernel_spmd

    if _NC_CACHE is None:
        _NC_CACHE = _build_nc()
    nc = _NC_CACHE

    in_maps = _host_inputs(q_A, k_A, q_mask, k_mask)
    res = run_bass_kernel_spmd(
        nc, in_maps, core_ids=list(range(NCORES)), trace=TRACE
    )
    _LAST = res

    alpha = np.empty((H, B, N, LK), np.float32)
    for core in range(NCORES):
        o = res.results[core]["out"]
        for u in range(PAIRS_PER_CORE):
            P = PAIRS_PER_CORE * core + u
            alpha[P // B, P % B] = o[u * N : (u + 1) * N]
    return alpha


# revision 4
# speedup vs baseline: 1.9716x; 1.1755x over previous
"""Trainium2 Bass kernel for nn_AttnCoef (sparse attention coefficients).

Problem: alpha = softmax_masked(q @ k^T / sqrt(DH)) over Lk = n^2, with an
all-distinct index mask M(i,(j,k)) = [i!=j][i!=k][j!=k] and node-validity
masks. Output [H=4, B=4, Lq=128, Lk=16384] f32 (128 MiB).

Strategy (8 NeuronCores, data parallel over the 16 (h,b) pairs, 2 per core):
- All masking is folded into the matmul as additive -C biases so that
  exp() underflows masked entries to exactly 0:
    * lk-only mask ([j!=k] & node masks) rides an extra contraction row
      (weight 1.0, bias row -C*(1-kvalid)).
    * [i=j] block mask: 4 extra contraction rows with per-chunk one-hot
      weights (chunk = 512 lk = 4 j-blocks).
    * [i=k] strided diagonal mask: second accumulating matmul
      (-C*I as stationary, periodic identity as moving operand).
- One ScalarE pass: p = exp(0.25*psum) PSUM->SBUF (bf16) with fused
  per-row accumulation (denominators).
- recip = q_mask / (denom + eps); one VectorE tensor_scalar pass scales
  p by recip into f32; DMA out.
"""

import sys

sys.path.insert(0, "/opt/trn_rl_repo")

import numpy as np
import ml_dtypes

H, B, N, DQK, DH = 4, 4, 128, 64, 16
LK = N * N  # 16384
NCORES = 8
PAIRS_PER_CORE = 2
NCHUNK, CW = 32, 512  # matmul chunks per pair
NGRP, GW = 8, 2048  # psum groups per pair (4 chunks each)
NOUT, OW = 4, 4096  # output store chunks per pair
KDIM = DH + 1 + 4  # 21 contraction rows
BIGC = 98304.0  # additive mask constant (exact in bf16/f32)

TRACE = False
_LAST = None
_NC_CACHE = None


def _build_nc():
    import concourse.tile as tile
    from concourse import bacc, mybir

    nc = bacc.Bacc(None, target_bir_lowering=False)
    f32, bf16 = mybir.dt.float32, mybir.dt.bfloat16

    lhs_e = nc.declare_dram_parameter(
        "lhs", [KDIM, PAIRS_PER_CORE, NCHUNK, N], bf16, isOutput=False
    )
    rhs_e = nc.declare_dram_parameter(
        "rhs", [PAIRS_PER_CORE, KDIM, LK], bf16, isOutput=False
    )
    wid_e = nc.declare_dram_parameter("wid", [N, N], bf16, isOutput=False)
    irep_e = nc.declare_dram_parameter("irep", [N, CW], bf16, isOutput=False)
    qm_e = nc.declare_dram_parameter("qmask", [N, PAIRS_PER_CORE], f32, isOutput=False)
    out_e = nc.declare_dram_parameter(
        "out", [PAIRS_PER_CORE * N, LK], f32, isOutput=True
    )

    EXP = mybir.ActivationFunctionType.Exp
    ADD = mybir.AluOpType.add
    AXX = mybir.AxisListType.X

    with tile.TileContext(nc) as tc:
        with (
            tc.tile_pool(name="consts", bufs=1) as consts,
            tc.tile_pool(name="rhsp", bufs=2) as rhsp,
            tc.tile_pool(name="pp", bufs=2) as pp,
            tc.tile_pool(name="psum", bufs=2, space="PSUM") as psum,
            tc.tile_pool(name="small", bufs=2) as small,
            tc.tile_pool(name="op", bufs=2) as op,
        ):
            lhs_t = consts.tile([KDIM, PAIRS_PER_CORE, NCHUNK, N], bf16)
            nc.sync.dma_start(out=lhs_t[:], in_=lhs_e[:])
            wid_t = consts.tile([N, N], bf16)
            nc.sync.dma_start(out=wid_t[:], in_=wid_e[:])
            irep_t = consts.tile([N, CW], bf16)
            nc.sync.dma_start(out=irep_t[:], in_=irep_e[:])
            qm_t = consts.tile([N, PAIRS_PER_CORE], f32)
            nc.sync.dma_start(out=qm_t[:], in_=qm_e[:])

            out_ap = out_e[:]

            # HAM warm-up: the real MM mix (K=21 alternating with K=128)
            # never trips the PE activity monitor, leaving the clock gated
            # at 1.2 GHz for the whole kernel. A burst of full-array
            # matmuls here (overlapping the rhs DMA) un-throttles it.
            for w in range(16):
                wps = psum.tile([N, GW], f32, tag="ps", name=f"wps{w}")
                nc.tensor.matmul(
                    wps[:, :CW], wid_t[:], irep_t[:], start=True, stop=True
                )
            wsink = consts.tile([N, 1], f32)
            nc.vector.tensor_reduce(
                out=wsink, in_=wps[:, :CW], axis=AXX, op=ADD
            )

            for u in range(PAIRS_PER_CORE):
                rhs_t = rhsp.tile([KDIM, LK], bf16, tag="rhs")
                nc.sync.dma_start(out=rhs_t[:], in_=rhs_e[:][u])

                p_t = pp.tile([N, LK], bf16, tag="p")
                dsum = small.tile([N, NGRP], f32, tag="dsum")

                for g in range(NGRP):
                    ps = psum.tile([N, GW], f32, tag="ps")
                    for cc in range(4):
                        c = 4 * g + cc
                        nc.tensor.matmul(
                            ps[:, cc * CW : (cc + 1) * CW],
                            lhs_t[:, u, c, :],
                            rhs_t[:, c * CW : (c + 1) * CW],
                            start=True,
                            stop=False,
                        )
                    for cc in range(4):
                        nc.tensor.matmul(
                            ps[:, cc * CW : (cc + 1) * CW],
                            wid_t[:],
                            irep_t[:],
                            start=False,
                            stop=True,
                        )
                    nc.scalar.activation(
                        out=p_t[:, g * GW : (g + 1) * GW],
                        in_=ps[:],
                        func=EXP,
                        scale=0.25,
                        accum_out=dsum[:, g : g + 1],
                    )

                den = small.tile([N, 1], f32, tag="den")
                nc.vector.tensor_reduce(out=den, in_=dsum[:], axis=AXX, op=ADD)
                den2 = small.tile([N, 1], f32, tag="den2")
                nc.vector.tensor_scalar_add(out=den2, in0=den, scalar1=1e-30)
                recip = small.tile([N, 1], f32, tag="recip")
                nc.vector.reciprocal(out=recip, in_=den2)
                recipf = small.tile([N, 1], f32, tag="recipf")
                nc.vector.tensor_mul(out=recipf, in0=recip, in1=qm_t[:, u : u + 1])

                for g in range(NOUT):
                    ob = op.tile([N, OW], f32, tag="ob")
                    nc.vector.tensor_scalar_mul(
                        out=ob, in0=p_t[:, g * OW : (g + 1) * OW], scalar1=recipf
                    )
                    nc.sync.dma_start(
                        out=out_ap[u * N : (u + 1) * N, g * OW : (g + 1) * OW],
                        in_=ob[:],
                    )

    nc.compile()
    return nc


def _host_inputs(q_A, k_A, q_mask, k_mask):
    q_A = np.ascontiguousarray(np.asarray(q_A, dtype=np.float32))
    k_A = np.ascontiguousarray(np.asarray(k_A, dtype=np.float32))
    q_mask = np.asarray(q_mask).astype(bool)
    k_mask = np.asarray(k_mask).astype(bool)

    # [h, b, d, i] and [h, b, d, lk]
    qt = q_A.reshape(B, N, H, DH).transpose(2, 0, 3, 1)
    kt = k_A.reshape(B, LK, H, DH).transpose(2, 0, 3, 1)

    jne = ~np.eye(N, dtype=bool)
    kvalid = (k_mask & jne[None]).reshape(B, LK)  # [b, lk]
    row16 = (-BIGC) * (~kvalid).astype(np.float32)  # [b, lk]

    lk = np.arange(LK)
    # j-block bias rows (periodic in chunks of 512): -C where (lk//128)%4 == t
    jpat = np.where(
        ((lk // N) % 4)[None, :] == np.arange(4)[:, None], -BIGC, 0.0
    ).astype(np.float32)  # [4, LK]
    # per-chunk one-hot weights for the j rows: 1.0 where i == 4c+t
    ii = np.arange(N)
    cidx = np.arange(NCHUNK)
    lhsaug = (
        (ii[None, None, :] == (4 * cidx[None, :, None] + np.arange(4)[:, None, None]))
    ).astype(np.float32)  # [4, 32, 128]

    wid = (-BIGC * np.eye(N, dtype=np.float32)).astype(ml_dtypes.bfloat16)
    irep = (np.arange(CW) % N == np.arange(N)[:, None]).astype(ml_dtypes.bfloat16)

    in_maps = []
    for core in range(NCORES):
        lhs_arr = np.zeros((KDIM, PAIRS_PER_CORE, NCHUNK, N), ml_dtypes.bfloat16)
        rhs_arr = np.empty((PAIRS_PER_CORE, KDIM, LK), ml_dtypes.bfloat16)
        qm_arr = np.zeros((N, PAIRS_PER_CORE), np.float32)
        for u in range(PAIRS_PER_CORE):
            P = PAIRS_PER_CORE * core + u
            h, b = P // B, P % B
            lhs_arr[0:DH, u, :, :] = qt[h, b][:, None, :]
            lhs_arr[DH, u, :, :] = 1.0
            lhs_arr[DH + 1 :, u, :, :] = lhsaug
            rhs_arr[u, 0:DH] = kt[h, b]
            rhs_arr[u, DH] = row16[b]
            rhs_arr[u, DH + 1 :] = jpat
            qm_arr[:, u] = q_mask[b].astype(np.float32)
        in_maps.append(
            {
                "lhs": lhs_arr,
                "rhs": rhs_arr,
                "wid": wid,
                "irep": irep,
                "qmask": qm_arr,
            }
        )
    return in_maps


def kernel(q_A, k_A, q_mask, k_mask):
    global _NC_CACHE, _LAST
    from concourse.bass_utils import run_bass_kernel_spmd

    if _NC_CACHE is None:
        _NC_CACHE = _build_nc()
    nc = _NC_CACHE

    in_maps = _host_inputs(q_A, k_A, q_mask, k_mask)
    res = run_bass_kernel_spmd(
        nc, in_maps, core_ids=list(range(NCORES)), trace=TRACE
    )
    _LAST = res

    alpha = np.empty((H, B, N, LK), np.float32)
    for core in range(NCORES):
        o = res.results[core]["out"]
        for u in range(PAIRS_PER_CORE):
            P = PAIRS_PER_CORE * core + u
            alpha[P // B, P % B] = o[u * N : (u + 1) * N]
    return alpha
